# revision 1
# baseline (speedup 1.0000x reference)
"""KAN layer kernel for Trainium2, data-parallel over 8 NeuronCores.

Math: out[b,o] = sum_i comb_w[i,o] * (w1*x + w2*x^2 + w3*x^3 + edge_b)[b,i,o]
    = x @ W1 + x^2 @ W2 + x^3 @ W3 + bias
  where Wp[i,o] = edge_w[i,o,p] * comb_w[i,o],  bias[o] = sum_i comb_w[i,o]*edge_b[i,o].

Stacking the basis along the contraction dim turns this into one matmul:
  out = [x | x^2 | x^3] @ vstack(W1, W2, W3)   # [B,1536] @ [1536,512]

Sharding: batch 8-way (1024 rows/core), weights replicated. Each core
receives x transposed ([512,1024], contraction on partitions), computes
x^2/x^3 on-chip, accumulates 12 K-chunks into 8 PSUM banks (4 o-tiles x
2 batch halves), adds bias during PSUM->SBUF copy, writes out^T
[512,1024]. Host gathers and transposes back.

fp32r is the full-rate fp32 matmul mode; its fused LDW+MM encoding (and
the HWDGE DMA descriptor) only admit ONE semaphore wait, so:
- exactly 8 HWDGE DMAs are issued (bias, 3 weight thirds, 2 x halves,
  2 output halves) -> every DMA gets a fresh lane, no lane-FIFO waits;
- tiny "gate" matmuls absorb each basis tile's producer wait on the PE
  queue (same-x-DMA operands for x, an all-zero stationary operand for
  x^2/x^3 so their accumulated contribution is exactly zero);
- each weight third is aligned with one basis power, so the first
  matmul of a power carries just that weight DMA's wait.
PE instruction order is pinned with sync=False dep edges.
"""

import sys

import numpy as np

sys.path.insert(0, "/opt/trn_rl_repo")

import concourse.bass as bass
import concourse.tile as tile
from concourse import bass_utils, mybir
from concourse.tile_rust import add_dep_helper

B, I, O = 8192, 512, 512
NCORES = 8
BS = B // NCORES  # 1024 rows per core
KT = (3 * I) // 128  # 12 contraction chunks of 128
PT = I // 128  # 4 partition tiles for I and O
NB = BS // 512  # 2 batch column chunks of 512 (PSUM bank width)

MM_DT = mybir.dt.float32r  # full-rate fp32 matmul mode

_nc = None


def _build():
    nc = bass.Bass("TRN2", target_bir_lowering=False, debug=False)
    f32 = mybir.dt.float32
    xt = nc.dram_tensor("xt", [I, BS], MM_DT, kind="ExternalInput")
    w = nc.dram_tensor("w", [3 * I + 128, O], MM_DT, kind="ExternalInput")
    yt = nc.dram_tensor("yt", [O, BS], f32, kind="ExternalOutput")

    xt_r = xt.ap().rearrange("(t p) b -> p t b", p=128)  # [128, 4, 1024]
    w_r = w.ap().rearrange("(t p) o -> p t o", p=128)  # [128, 13, 512]
    yt_r = yt.ap().rearrange("(t p) b -> p t b", p=128)  # [128, 4, 1024]

    pe_chain = []  # forced PE program order (sync=False edges)

    def pe(inst):
        if pe_chain:
            add_dep_helper(inst.ins, pe_chain[-1].ins, sync=False, reason="pe order")
        pe_chain.append(inst)
        return inst

    with tile.TileContext(nc) as tc:
        with (
            tc.tile_pool(name="consts", bufs=1) as cpool,
            tc.tile_pool(name="acts", bufs=1) as apool,
            tc.tile_pool(name="out", bufs=1) as opool,
            tc.tile_pool(name="psum", bufs=1, space="PSUM") as pspool,
        ):
            zeros_sb = apool.tile([128, 2], MM_DT)

            # one weight DMA per basis power (k-chunks 4g..4g+3); the bias
            # tiles ride as a 13th chunk in the last one, so only 6 HWDGE
            # DMAs exist in total -> every DMA gets a fresh lane (no lane
            # waits) and the tail drain stays within its 8-wait budget
            w_sb = cpool.tile([128, KT + 1, O], MM_DT)
            for g in range(3):
                hi = 4 * g + 4 if g < 2 else KT + 1
                nc.sync.dma_start(
                    out=w_sb[:, 4 * g : hi, :], in_=w_r[:, 4 * g : hi, :]
                )
            # absorb the bias DMA wait on DVE early; bias-adds then reach
            # bias_gate through the in-order DVE queue with no extra wait
            bias_gate = cpool.tile([128, PT], f32)
            nc.vector.tensor_copy(bias_gate, w_sb[:, KT, 0:PT].bitcast(f32))

            x_sb = apool.tile([128, PT, BS], MM_DT)
            x2_sb = apool.tile([128, PT, BS], MM_DT)
            x3_sb = apool.tile([128, PT, BS], MM_DT)
            nc.sync.dma_start(out=x_sb, in_=xt_r)
            # all-zero fp32r stationary operand for the x^2/x^3 gates
            # (memset can't produce fp32r; a *0.0 DVE op can)
            nc.vector.tensor_scalar_mul(zeros_sb, x_sb[:, 0, 0:2], 0.0)
            for n in range(NB):
                cs = slice(n * 512, (n + 1) * 512)
                nc.vector.tensor_mul(x2_sb[:, :, cs], x_sb[:, :, cs], x_sb[:, :, cs])
                nc.vector.tensor_mul(x3_sb[:, :, cs], x2_sb[:, :, cs], x_sb[:, :, cs])

            basis = [x_sb, x2_sb, x3_sb]
            ps = [
                pspool.tile([128, 512], f32, name=f"ps{i}", tag=f"ps{i}")
                for i in range(NB * PT)
            ]

            def gate(p_, n):
                # 1-wait matmul absorbing basis[p_] chunk n's producer dep
                # before the real (fused-LDW fp32r) matmuls read it. For
                # p_=0 both operands come from the same x DMA (one wait,
                # garbage result into a not-yet-started psum bank). For
                # p_>0 the stationary operand is all zeros (DVE memset,
                # same DVE semaphore as the x^p producer -> one wait) so
                # the accumulated contribution is exactly 0.
                s = slice(n * 512, n * 512 + 2)
                lhsT = x_sb[:, 0, s] if p_ == 0 else zeros_sb
                return pe(
                    nc.tensor.matmul(
                        ps[n * PT][0:2, 0:2],
                        lhsT,
                        basis[p_][:, 0, s],
                        start=(p_ == 0),
                        stop=(p_ == 0),
                        skip_group_check=True,
                    )
                )

            y_sb = opool.tile([128, PT, BS], f32)
            for n in range(NB):
                cs = slice(n * 512, (n + 1) * 512)
                for p_, k in enumerate(range(0, KT, PT)):
                    gate(p_, n)
                    for kk in range(k, k + PT):
                        for to in range(PT):
                            pe(
                                nc.tensor.matmul(
                                    ps[n * PT + to],
                                    w_sb[:, kk, to * 128 : (to + 1) * 128],
                                    basis[p_][:, kk % PT, cs],
                                    start=(kk == 0),
                                    stop=(kk == KT - 1),
                                    skip_group_check=(kk == 0 or kk == KT - 1),
                                )
                            )
                for to in range(PT):
                    nc.vector.tensor_scalar_add(
                        y_sb[:, to, cs], ps[n * PT + to], bias_gate[:, to : to + 1]
                    )
                nc.sync.dma_start(out=yt_r[:, :, cs], in_=y_sb[:, :, cs])

    # Post-pass: walrus codegen admits only one sync-wait per instruction
    # encoding here; Tile's kernel-tail drain aggregates one wait per
    # outstanding semaphore. Split any multi-wait instruction into a chain
    # of single-wait drains ahead of it on the same engine queue.
    for bb in nc.m.functions[0].blocks:
        insts = list(bb.instructions)
        out, split = [], 0
        for ins in insts:
            si = ins.sync_info
            waits = list(si.on_wait) if si and si.on_wait else []
            if len(waits) > 1:
                for wx in waits[:-1]:
                    nd = mybir.InstDrain(
                        name=f"drain_split_{split}", engine=ins.engine
                    )
                    split += 1
                    nd.sync_info = mybir.SyncInfo(on_wait=[wx], on_update=[])
                    out.append(nd)
                si.on_wait = [waits[-1]]
            out.append(ins)
        if split:
            bb.set_instructions_from_list(out) if hasattr(
                bb, "set_instructions_from_list"
            ) else setattr(bb, "instructions", out)
    return nc


last_results = None  # BassKernelResults of the most recent run (for test harness)


def kernel(x, edge_w, edge_b, comb_w):
    global _nc, last_results
    if _nc is None:
        _nc = _build()

    w_eff = (edge_w * comb_w[:, :, None]).astype(np.float32)  # [I, O, 3]
    w_big = np.ascontiguousarray(
        np.concatenate([w_eff[:, :, 0], w_eff[:, :, 1], w_eff[:, :, 2]], axis=0)
    )  # [1536, O]
    bias = np.sum(comb_w * edge_b, axis=0, dtype=np.float64).astype(np.float32)
    pad = np.zeros((128, O), np.float32)
    pad[:, :PT] = bias.reshape(PT, 128).T  # bias tiles as 13th weight chunk
    w_big = np.ascontiguousarray(np.concatenate([w_big, pad], axis=0))

    in_maps = []
    for c in range(NCORES):
        xs = np.ascontiguousarray(x[c * BS : (c + 1) * BS].T)  # [I, BS]
        in_maps.append({"xt": xs, "w": w_big})

    res = bass_utils.run_bass_kernel_spmd(_nc, in_maps, list(range(NCORES)))
    last_results = res
    out = np.concatenate(
        [np.ascontiguousarray(res.results[c]["yt"]).T for c in range(NCORES)], axis=0
    )
    return out



# revision 3
# speedup vs baseline: 1.5677x; 1.5677x over previous
"""KAN layer kernel for Trainium2, data-parallel over 8 NeuronCores.

Math: out[b,o] = sum_i comb_w[i,o] * (w1*x + w2*x^2 + w3*x^3 + edge_b)[b,i,o]
    = x @ W1 + x^2 @ W2 + x^3 @ W3 + bias
  where Wp[i,o] = edge_w[i,o,p] * comb_w[i,o],  bias[o] = sum_i comb_w[i,o]*edge_b[i,o].

Stacked along the contraction dim this is one [B,1536] @ [1536,512] matmul.

Sharding: batch 8-way (1024 rows/core), weights replicated. Everything is
bf16 on the wire and in the matmuls (fp32 PSUM accumulation): bf16 runs the
PE at full rate (1 cyc/row vs ~2 for fp32r on HW) and halves DMA bytes.
Verified numerics: max rel err ~6e-3 vs the fp32 reference (gate is 2e-2).

Per core:
- x^T arrives as [512, 1024] bf16; weights as 13 chunks of [128, 512] bf16
  in consumption order (chunk 3t+p = basis power p of k-tile t; chunk 12
  carries the fp32 bias bit-packed into bf16 pairs).
- DMA is split across both HWDGE queues: x + output on the sync queue,
  weights + bias on the scalar queue. x and w are fetched per k-tile so
  matmuls start after the first tile lands, not after the whole tensor.
- DVE computes x^2/x^3 per tile as it arrives.
- 48 matmuls of [128k,128o] x [128k, 1024b] (bf16 moving max), PSUM as
  4 tiles of [128, 1024] f32 = all 8 banks; accumulate 12 chunks each.
  Phase A (o-tiles 0,1) is t-major to pipeline with the x DMA; phase B
  (o-tiles 2,3) is o-major so each o-tile's output drains (bias add on
  DVE + DMA out) while the next one is still accumulating.
- Output is written as y^T [512, 1024] bf16; host transposes/casts back.
"""

import sys

import numpy as np
import ml_dtypes

sys.path.insert(0, "/opt/trn_rl_repo")

import concourse.bass as bass
import concourse.tile as tile
from concourse import bass_utils, mybir
from concourse.tile_rust import add_dep_helper

B, I, O = 8192, 512, 512
NCORES = 8
BS = B // NCORES  # 1024 rows per core
PT = 4  # 128-row tiles in I (k-tiles) and O (o-tiles)
NPOW = 3  # basis powers: x, x^2, x^3
NCHUNK = NPOW * PT  # 12 contraction chunks of 128
WROWS = (NCHUNK + 1) * 128  # 12 weight chunks + 1 bias chunk

BF = mybir.dt.bfloat16
F32 = mybir.dt.float32

_nc = None


def _build():
    nc = bass.Bass("TRN2", target_bir_lowering=False, debug=False)
    xt = nc.dram_tensor("xt", [I, BS], BF, kind="ExternalInput")
    w = nc.dram_tensor("w", [WROWS, O], BF, kind="ExternalInput")
    yt = nc.dram_tensor("yt", [O, BS], BF, kind="ExternalOutput")

    xt_r = xt.ap().rearrange("(t p) b -> p t b", p=128)  # [128, 4, 1024]
    w_r = w.ap().rearrange("(c q) o -> q c o", q=128)  # [128, 13, 512]
    yt_r = yt.ap().rearrange("(t p) b -> p t b", p=128)  # [128, 4, 1024]

    pe_chain = []  # forced PE program order (sync=False edges)

    def pe(inst):
        if pe_chain:
            add_dep_helper(inst.ins, pe_chain[-1].ins, sync=False, reason="pe order")
        pe_chain.append(inst)
        return inst

    with tile.TileContext(nc) as tc:
        with (
            tc.tile_pool(name="consts", bufs=1) as cpool,
            tc.tile_pool(name="acts", bufs=1) as apool,
            tc.tile_pool(name="out", bufs=1) as opool,
            tc.tile_pool(name="psum", bufs=1, space="PSUM") as pspool,
        ):
            w_sb = cpool.tile([128, NCHUNK + 1, O], BF)
            x_sb = apool.tile([128, PT, BS], BF)
            x2_sb = apool.tile([128, PT, BS], BF)
            x3_sb = apool.tile([128, PT, BS], BF)
            y_sb = opool.tile([128, PT, BS], BF)

            # bias: chunk 12, cols 0..7 hold [128,4] f32 bit-packed as bf16 pairs
            bias_f32 = w_sb[:, NCHUNK, 0:8].bitcast(F32)  # [128, 4]

            # per k-tile: x tile on sync queue, weight chunks 3t..3t+2 on
            # scalar queue; DVE squares/cubes the tile as it lands
            for t in range(PT):
                nc.sync.dma_start(out=x_sb[:, t, :], in_=xt_r[:, t, :])
                nc.scalar.dma_start(
                    out=w_sb[:, 3 * t : 3 * t + 3, :], in_=w_r[:, 3 * t : 3 * t + 3, :]
                )
                nc.vector.tensor_mul(x2_sb[:, t, :], x_sb[:, t, :], x_sb[:, t, :])
                nc.vector.tensor_mul(x3_sb[:, t, :], x2_sb[:, t, :], x_sb[:, t, :])
            # bias chunk rides the scalar queue after the weights
            nc.scalar.dma_start(
                out=w_sb[:, NCHUNK : NCHUNK + 1, :], in_=w_r[:, NCHUNK : NCHUNK + 1, :]
            )

            basis = [x_sb, x2_sb, x3_sb]
            # 8 PSUM banks: ps[n*4+o] = batch half n, o-tile o, [128, 512] f32
            ps = [
                pspool.tile([128, 512], F32, name=f"ps{i}", tag=f"ps{i}")
                for i in range(2 * PT)
            ]

            def mm(n, o, t, p):
                pe(
                    nc.tensor.matmul(
                        ps[n * PT + o],
                        w_sb[:, 3 * t + p, o * 128 : (o + 1) * 128],
                        basis[p][:, t, n * 512 : (n + 1) * 512],
                        start=(t == 0 and p == 0),
                        stop=(t == PT - 1 and p == NPOW - 1),
                    )
                )

            def copy_out(n, o):
                # PSUM -> SBUF with bias add (f32 -> bf16)
                nc.vector.tensor_scalar_add(
                    y_sb[:, o, n * 512 : (n + 1) * 512],
                    ps[n * PT + o],
                    bias_f32[:, o : o + 1],
                )

            # phase A: batch half 0, t-major (pipelines with x arrival)
            for t in range(PT):
                for p in range(NPOW):
                    for o in range(PT):
                        mm(0, o, t, p)
            # phase B: batch half 1, o-major (early per-o drain).
            # half-0 copies run on DVE as soon as phase A's banks stop.
            for o in range(PT):
                copy_out(0, o)
            for o in range(PT):
                for t in range(PT):
                    for p in range(NPOW):
                        mm(1, o, t, p)
                copy_out(1, o)
                # both halves of o-tile o are in y_sb now; ship it
                queue = nc.scalar if o % 2 == 0 else nc.sync
                queue.dma_start(out=yt_r[:, o, :], in_=y_sb[:, o, :])

    # Post-pass: walrus codegen admits only one sync-wait per instruction
    # encoding here; Tile's kernel-tail drain aggregates one wait per
    # outstanding semaphore. Split any multi-wait instruction into a chain
    # of single-wait drains ahead of it on the same engine queue.
    for bb in nc.m.functions[0].blocks:
        insts = list(bb.instructions)
        out, split = [], 0
        for ins in insts:
            si = ins.sync_info
            waits = list(si.on_wait) if si and si.on_wait else []
            if len(waits) > 1:
                for wx in waits[:-1]:
                    nd = mybir.InstDrain(
                        name=f"drain_split_{split}", engine=ins.engine
                    )
                    split += 1
                    nd.sync_info = mybir.SyncInfo(on_wait=[wx], on_update=[])
                    out.append(nd)
                si.on_wait = [waits[-1]]
            out.append(ins)
        if split:
            bb.set_instructions_from_list(out) if hasattr(
                bb, "set_instructions_from_list"
            ) else setattr(bb, "instructions", out)
    return nc


last_results = None  # BassKernelResults of the most recent run (for test harness)


def kernel(x, edge_w, edge_b, comb_w):
    global _nc, last_results
    if _nc is None:
        _nc = _build()

    bf16 = ml_dtypes.bfloat16
    w_eff = (edge_w * comb_w[:, :, None]).astype(np.float32)  # [I, O, 3]
    # chunk c = 3t+p: rows 128t..128(t+1) of W_p, in matmul consumption order
    w_big = np.empty((WROWS, O), dtype=bf16)
    for t in range(PT):
        for p in range(NPOW):
            w_big[(3 * t + p) * 128 : (3 * t + p + 1) * 128] = w_eff[
                t * 128 : (t + 1) * 128, :, p
            ].astype(bf16)
    # bias chunk: [128,4] f32 bit-packed into bf16 pairs at cols 0..7
    bias = np.sum(comb_w * edge_b, axis=0, dtype=np.float64).astype(np.float32)
    pad = np.zeros((128, O), dtype=bf16)
    pad_u16 = pad.view(np.uint16)
    pad_u16[:, :8] = np.ascontiguousarray(bias.reshape(PT, 128).T).view(np.uint16)
    w_big[NCHUNK * 128 :] = pad

    in_maps = []
    for c in range(NCORES):
        xs = np.ascontiguousarray(x[c * BS : (c + 1) * BS].T.astype(bf16))  # [I, BS]
        in_maps.append({"xt": xs, "w": w_big})

    res = bass_utils.run_bass_kernel_spmd(_nc, in_maps, list(range(NCORES)))
    last_results = res
    out = np.concatenate(
        [
            np.asarray(res.results[c]["yt"]).astype(np.float32).T
            for c in range(NCORES)
        ],
        axis=0,
    )
    return out


# revision 5
# speedup vs baseline: 1.6090x; 1.0264x over previous
"""KAN layer kernel for Trainium2, data-parallel over 8 NeuronCores.

Math: out[b,o] = sum_i comb_w[i,o] * (w1*x + w2*x^2 + w3*x^3 + edge_b)[b,i,o]
    = x @ W1 + x^2 @ W2 + x^3 @ W3 + bias
  where Wp[i,o] = edge_w[i,o,p] * comb_w[i,o],  bias[o] = sum_i comb_w[i,o]*edge_b[i,o].

Stacked along the contraction dim this is one [B,1536] @ [1536,512] matmul.

Sharding: batch 8-way (1024 rows/core), weights replicated. Everything is
bf16 on the wire and in the matmuls (fp32 PSUM accumulation): bf16 runs the
PE at full rate (1 cyc/row vs ~2 for fp32r on HW) and halves DMA bytes.
Verified numerics: max rel err ~6e-3 vs the fp32 reference (gate is 2e-2).

Per core:
- x^T arrives as [512, 1024] bf16; weights as 13 chunks of [128, 512] bf16
  in consumption order (chunk 3t+p = basis power p of k-tile t; chunk 12
  carries the fp32 bias bit-packed into bf16 pairs).
- DMA is split across both HWDGE queues: x + output on the sync queue,
  weights + bias on the scalar queue. x and w are fetched per k-tile so
  matmuls start after the first tile lands, not after the whole tensor.
- DVE computes x^2/x^3 per tile as it arrives.
- 48 matmuls of [128k,128o] x [128k, 1024b] (bf16 moving max), PSUM as
  4 tiles of [128, 1024] f32 = all 8 banks; accumulate 12 chunks each.
  Phase A (o-tiles 0,1) is t-major to pipeline with the x DMA; phase B
  (o-tiles 2,3) is o-major so each o-tile's output drains (bias add on
  DVE + DMA out) while the next one is still accumulating.
- Output is written as y^T [512, 1024] bf16; host transposes/casts back.
"""

import sys

import numpy as np
import ml_dtypes

sys.path.insert(0, "/opt/trn_rl_repo")

import concourse.bass as bass
import concourse.tile as tile
from concourse import bass_utils, mybir
from concourse.tile_rust import add_dep_helper

B, I, O = 8192, 512, 512
NCORES = 8
BS = B // NCORES  # 1024 rows per core
PT = 4  # 128-row tiles in I (k-tiles) and O (o-tiles)
NPOW = 3  # basis powers: x, x^2, x^3
NCHUNK = NPOW * PT  # 12 contraction chunks of 128
WROWS = (NCHUNK + 1) * 128  # 12 weight chunks + 1 bias chunk

BF = mybir.dt.bfloat16
F32 = mybir.dt.float32

_nc = None


def _build():
    nc = bass.Bass("TRN2", target_bir_lowering=False, debug=False)
    xt = nc.dram_tensor("xt", [I, BS], BF, kind="ExternalInput")
    w = nc.dram_tensor("w", [WROWS, O], BF, kind="ExternalInput")
    yt = nc.dram_tensor("yt", [O, BS], BF, kind="ExternalOutput")

    xt_r = xt.ap().rearrange("(t p) b -> p t b", p=128)  # [128, 4, 1024]
    w_r = w.ap().rearrange("(c q) o -> q c o", q=128)  # [128, 13, 512]
    yt_r = yt.ap().rearrange("(t p) b -> p t b", p=128)  # [128, 4, 1024]

    pe_chain = []  # forced PE program order (sync=False edges)

    def pe(inst):
        if pe_chain:
            add_dep_helper(inst.ins, pe_chain[-1].ins, sync=False, reason="pe order")
        pe_chain.append(inst)
        return inst

    # HAM warm-up: ~3.4us of dummy matmuls on garbage SBUF in the main
    # block, so the PE clock gate is already at 8/8 (2.4 GHz) when the
    # real matmuls start. The scratch PSUM bank is freed before the tile
    # pools allocate; real banks are zeroed by their start=True matmuls.
    warm_w = nc.alloc_sbuf_tensor("warm_w", [128, 2], BF)
    warm_x = nc.alloc_sbuf_tensor("warm_x", [128, 512], BF)
    with nc.psum_tensor("warm_ps", [128, 512], F32) as wps:
        for i in range(8):
            nc.tensor.matmul(
                wps.ap()[0:2, :], warm_w.ap(), warm_x.ap(), start=True, stop=True
            )

    with tile.TileContext(nc) as tc:
        with (
            tc.tile_pool(name="consts", bufs=1) as cpool,
            tc.tile_pool(name="acts", bufs=1) as apool,
            tc.tile_pool(name="out", bufs=1) as opool,
            tc.tile_pool(name="psum", bufs=1, space="PSUM") as pspool,
        ):
            w_sb = cpool.tile([128, NCHUNK + 1, O], BF)
            x_sb = apool.tile([128, PT, BS], BF)
            x2_sb = apool.tile([128, PT, BS], BF)
            x3_sb = apool.tile([128, PT, BS], BF)
            y_sb = opool.tile([128, PT, BS], BF)

            # bias: chunk 12, cols 0..7 hold [128,4] f32 bit-packed as bf16 pairs
            bias_f32 = w_sb[:, NCHUNK, 0:8].bitcast(F32)  # [128, 4]

            # x arrives per (k-tile, batch-half) on the sync queue, half 0
            # (phase A's operands) first; DVE squares/cubes each piece as
            # it lands. Weights ride the scalar queue: chunk 0 alone (it
            # gates the first matmul), then coarser groups, bias last.
            def halfs(h):
                return slice(h * 512, (h + 1) * 512)

            for h in range(2):
                for t in range(PT):
                    nc.sync.dma_start(
                        out=x_sb[:, t, halfs(h)], in_=xt_r[:, t, halfs(h)]
                    )
                    nc.vector.tensor_mul(
                        x2_sb[:, t, halfs(h)], x_sb[:, t, halfs(h)], x_sb[:, t, halfs(h)]
                    )
                    nc.vector.tensor_mul(
                        x3_sb[:, t, halfs(h)], x2_sb[:, t, halfs(h)], x_sb[:, t, halfs(h)]
                    )
            nc.scalar.dma_start(out=w_sb[:, 0:1, :], in_=w_r[:, 0:1, :])
            nc.scalar.dma_start(out=w_sb[:, 1:3, :], in_=w_r[:, 1:3, :])
            for t in range(1, PT):
                nc.scalar.dma_start(
                    out=w_sb[:, 3 * t : 3 * t + 3, :], in_=w_r[:, 3 * t : 3 * t + 3, :]
                )
            # bias chunk rides the scalar queue after the weights
            nc.scalar.dma_start(
                out=w_sb[:, NCHUNK : NCHUNK + 1, :], in_=w_r[:, NCHUNK : NCHUNK + 1, :]
            )

            basis = [x_sb, x2_sb, x3_sb]
            # 8 PSUM banks: ps[n*4+o] = batch half n, o-tile o, [128, 512] f32
            ps = [
                pspool.tile([128, 512], F32, name=f"ps{i}", tag=f"ps{i}")
                for i in range(2 * PT)
            ]

            def mm(n, o, t, p):
                pe(
                    nc.tensor.matmul(
                        ps[n * PT + o],
                        w_sb[:, 3 * t + p, o * 128 : (o + 1) * 128],
                        basis[p][:, t, n * 512 : (n + 1) * 512],
                        start=(t == 0 and p == 0),
                        stop=(t == PT - 1 and p == NPOW - 1),
                    )
                )

            def copy_out(n, o):
                # PSUM -> SBUF with bias add (f32 -> bf16)
                nc.vector.tensor_scalar_add(
                    y_sb[:, o, n * 512 : (n + 1) * 512],
                    ps[n * PT + o],
                    bias_f32[:, o : o + 1],
                )

            # phase A: batch half 0, t-major (pipelines with x arrival)
            for t in range(PT):
                for p in range(NPOW):
                    for o in range(PT):
                        mm(0, o, t, p)
            # phase B: batch half 1, o-major (early per-o drain).
            # half-0 copies run on DVE as soon as phase A's banks stop.
            for o in range(PT):
                copy_out(0, o)
            for o in range(PT):
                for t in range(PT):
                    for p in range(NPOW):
                        mm(1, o, t, p)
                copy_out(1, o)
                # both halves of o-tile o are in y_sb now; ship it
                queue = nc.scalar if o % 2 == 0 else nc.sync
                queue.dma_start(out=yt_r[:, o, :], in_=y_sb[:, o, :])

    # Post-pass: walrus codegen admits only one sync-wait per instruction
    # encoding here; Tile's kernel-tail drain aggregates one wait per
    # outstanding semaphore. Split any multi-wait instruction into a chain
    # of single-wait drains ahead of it on the same engine queue.
    for bb in nc.m.functions[0].blocks:
        insts = list(bb.instructions)
        out, split = [], 0
        for ins in insts:
            si = ins.sync_info
            waits = list(si.on_wait) if si and si.on_wait else []
            if len(waits) > 1:
                for wx in waits[:-1]:
                    nd = mybir.InstDrain(
                        name=f"drain_split_{split}", engine=ins.engine
                    )
                    split += 1
                    nd.sync_info = mybir.SyncInfo(on_wait=[wx], on_update=[])
                    out.append(nd)
                si.on_wait = [waits[-1]]
            out.append(ins)
        if split:
            bb.set_instructions_from_list(out) if hasattr(
                bb, "set_instructions_from_list"
            ) else setattr(bb, "instructions", out)
    return nc


last_results = None  # BassKernelResults of the most recent run (for test harness)


def kernel(x, edge_w, edge_b, comb_w):
    global _nc, last_results
    if _nc is None:
        _nc = _build()

    bf16 = ml_dtypes.bfloat16
    w_eff = (edge_w * comb_w[:, :, None]).astype(np.float32)  # [I, O, 3]
    # chunk c = 3t+p: rows 128t..128(t+1) of W_p, in matmul consumption order
    w_big = np.empty((WROWS, O), dtype=bf16)
    for t in range(PT):
        for p in range(NPOW):
            w_big[(3 * t + p) * 128 : (3 * t + p + 1) * 128] = w_eff[
                t * 128 : (t + 1) * 128, :, p
            ].astype(bf16)
    # bias chunk: [128,4] f32 bit-packed into bf16 pairs at cols 0..7
    bias = np.sum(comb_w * edge_b, axis=0, dtype=np.float64).astype(np.float32)
    pad = np.zeros((128, O), dtype=bf16)
    pad_u16 = pad.view(np.uint16)
    pad_u16[:, :8] = np.ascontiguousarray(bias.reshape(PT, 128).T).view(np.uint16)
    w_big[NCHUNK * 128 :] = pad

    in_maps = []
    for c in range(NCORES):
        xs = np.ascontiguousarray(x[c * BS : (c + 1) * BS].T.astype(bf16))  # [I, BS]
        in_maps.append({"xt": xs, "w": w_big})

    res = bass_utils.run_bass_kernel_spmd(_nc, in_maps, list(range(NCORES)))
    last_results = res
    out = np.concatenate(
        [
            np.asarray(res.results[c]["yt"]).astype(np.float32).T
            for c in range(NCORES)
        ],
        axis=0,
    )
    return out
